# revision 15
# baseline (speedup 1.0000x reference)
"""Trainium2 Bass kernel for GravityDisplacement (gnn_message_passing).

Data-parallel over batch B=8 across 8 NeuronCores (one sample per core).
Per core the full chain runs fused on-chip:

  MLP errors -> robust norm -> pairwise gravity/repulsion forces ->
  bounded displacement -> 3 iterations of error-aware density spreading.

Performance structure:
  * pairwise d2 is computed as (xi-xj)^2 + (yi-yj)^2 with ACT Square ops
    (per-partition bias = -xj) against GPSIMD-broadcast position rows --
    no PE matmul, no fp32 double-pass, exact fp32 differences.
  * all matmuls run in bf16 (single PE pass): MLP layers + transposes,
    and the L x L field reduction matmuls.
  * density phases only visit chunk pairs within +-2 grid rows
    (gaussian w < e^-40 beyond; guarded host-side by a grid check).
  * activations are emitted in function-grouped sweeps (Square lives in
    every ACT table, so the d2 squares never force a table reload);
    exp(relu(u)) is folded to max(exp(u),1).
  * diagonal zeroing via GPSIMD affine_select; global min/max/mean via
    GPSIMD partition_all_reduce; big DVE reciprocals via
    reciprocal_approx_fast; bf16 DVE temporaries (2x DVE rate).
"""

import os
import sys

sys.path.insert(0, "/opt/trn_rl_repo")

from contextlib import ExitStack

import numpy as np

import bass_rust
import concourse.bass as bass
import concourse.bacc as bacc
import concourse.tile as tile
from concourse import mybir
from concourse.bass_utils import run_bass_kernel_spmd
from concourse.masks import make_identity

AF = mybir.ActivationFunctionType
OP = mybir.AluOpType
AX = mybir.AxisListType
F32 = mybir.dt.float32
BF16 = mybir.dt.bfloat16

# ---- module constants (mirrors the nn.Module defaults) ----
N_ROW = 32
L = N_ROW * N_ROW            # 1024 latents
D = 256                      # latent_dim
H = 256                      # error_hidden_dim
SURF = 103.0
SPACING = SURF / (N_ROW - 1)
SMIN, SMAX = -SURF / 2, SURF / 2
DANGER = SPACING / 2.0
SIGMA = SPACING * 0.5
STEP = SPACING * 0.1
MAX_STEP = SPACING * 0.25
MAX_TOT = SPACING * 0.5
MAX_DISP, MIN_DISP = 3.0, 0.5
REPULSION = 0.5
DENSITY_ITERS = 3
S2 = 1.0 / (2.0 * SIGMA * SIGMA)   # gaussian exponent scale

P = 128                      # partitions
NCH = L // P                 # 8 chunks of i (and j-tiles)
B = 8                        # batch == n_cores
DENS_R = 2                   # density chunk radius (grid locality)

STAGE = int(os.environ.get("KSTAGE", "3"))


def _win(c, radius):
    if radius is None:
        return 0, NCH
    return max(0, c - radius), min(NCH, c + radius + 1)


def _build_kernel(ctx: ExitStack, tc: tile.TileContext, io: dict, radius):
    nc = tc.nc
    pos_d = io["positions"]
    out_d = io["out"]

    const = ctx.enter_context(tc.tile_pool(name="const", bufs=1))
    work = ctx.enter_context(tc.tile_pool(name="work", bufs=2))

    # ---------------- persistent tiles ----------------
    identity = const.tile([P, P], F32, name="identity")
    ident16 = const.tile([P, P], BF16, name="ident16")

    P_sb = const.tile([P, 2 * NCH], F32, name="P_sb")        # [p, (c,2)]
    P_neg = const.tile([P, 2 * NCH], F32, name="P_neg")
    P_start = const.tile([P, 2 * NCH], F32, name="P_start")
    Pw16 = const.tile([P, 3 * NCH], BF16, name="Pw16")       # [p, (c,3)] = x,y,1
    XY = const.tile([2, L], F32, name="XY")                  # x row, y row
    yrow = const.tile([1, L], F32, name="yrow")
    xb = const.tile([P, L], F32, name="xb")                  # x_i bcast
    yb = const.tile([P, L], F32, name="yb")

    lt16 = [const.tile([P, L], BF16, name=f"lt16_{k}") for k in range(2)]
    w1s = [const.tile([P, H], BF16, name=f"w1s{k}") for k in range(2)]
    w2s = [const.tile([P, H // 2], BF16, name=f"w2s{k}") for k in range(2)]
    w3s = const.tile([P, 1], BF16, name="w3s")
    b1r = const.tile([1, H], F32, name="b1r")
    lngr = const.tile([1, H], F32, name="lngr")
    lnbr = const.tile([1, H], F32, name="lnbr")
    b2r = const.tile([1, H // 2], F32, name="b2r")
    b3r = const.tile([1, 1], F32, name="b3r")
    b1b = const.tile([P, H], F32, name="b1b")
    lngb = const.tile([P, H], F32, name="lngb")
    lnbb = const.tile([P, H], F32, name="lnbb")
    b2b = const.tile([P, H // 2], F32, name="b2b")
    b3c = const.tile([P, 1], F32, name="b3c")

    anomS = const.tile([P, NCH], F32, name="anomS")          # 8 * (eln - mean)
    strength = const.tile([P, NCH], F32, name="strength")    # 1 - eln

    # ---------------- constant init ----------------
    make_identity(nc, identity[:])
    make_identity(nc, ident16[:])
    nc.gpsimd.memset(Pw16[:], 1.0)   # col 3c+2 stays 1 forever

    # ---------------- input DMA ----------------
    nc.sync.dma_start(
        out=P_sb[:].rearrange("p (c t) -> p c t", t=2),
        in_=pos_d.rearrange("(c p) t -> p c t", p=P),
    )
    for k in range(2):
        nc.sync.dma_start(out=lt16[k][:], in_=io["latT"][k * P:(k + 1) * P, :])
        nc.sync.dma_start(out=w1s[k][:], in_=io["w1"][k * P:(k + 1) * P, :])
        nc.sync.dma_start(out=w2s[k][:], in_=io["w2"][k * P:(k + 1) * P, :])
    nc.sync.dma_start(out=w3s[:], in_=io["w3"])
    nc.sync.dma_start(out=b1r[:], in_=io["b1"].unsqueeze(0))
    nc.sync.dma_start(out=lngr[:], in_=io["ln_g"].unsqueeze(0))
    nc.sync.dma_start(out=lnbr[:], in_=io["ln_b"].unsqueeze(0))
    nc.sync.dma_start(out=b2r[:], in_=io["b2"].unsqueeze(0))
    nc.sync.dma_start(out=b3r[:], in_=io["b3"].unsqueeze(0))

    # bias broadcasts on the GPSIMD engine
    nc.gpsimd.partition_broadcast(b1b[:], b1r[:])
    nc.gpsimd.partition_broadcast(lngb[:], lngr[:])
    nc.gpsimd.partition_broadcast(lnbb[:], lnbr[:])
    nc.gpsimd.partition_broadcast(b2b[:], b2r[:])
    nc.gpsimd.partition_broadcast(b3c[:], b3r[:])

    # ---------------- stage A: MLP errors (function-grouped sweeps) ------
    with tc.tile_pool(name="psumA", bufs=1, space="PSUM") as psA:
        pe_ = psA.tile([P, NCH], F32, name="pe_", tag="pe")

        xcs, isds = [], []
        for c in range(NCH):
            ph1 = psA.tile([P, H], F32, name="ph1", tag="h1", bufs=2)
            nc.tensor.matmul(ph1[:], lt16[0][:, c * P:(c + 1) * P], w1s[0][:],
                             start=True, stop=False)
            nc.tensor.matmul(ph1[:], lt16[1][:, c * P:(c + 1) * P], w1s[1][:],
                             start=False, stop=True)
            h1 = work.tile([P, H], F32, name="h1", tag="h1s", bufs=2)
            nc.vector.tensor_add(h1[:], ph1[:], b1b[:])
            mu = work.tile([P, 1], F32, name="mu", tag="mu", bufs=2)
            nc.vector.tensor_reduce(mu[:], h1[:], axis=AX.X, op=OP.add)
            mus = work.tile([P, 1], F32, name="mus", tag="mus", bufs=2)
            nc.scalar.mul(mus[:], mu[:], 1.0 / H)
            xc = work.tile([P, H], F32, name=f"xc{c}", tag=f"xc{c}", bufs=1)
            nc.vector.tensor_scalar_sub(xc[:], h1[:], mus[:])
            sq = work.tile([P, H], F32, name="sq", tag="sq", bufs=2)
            nc.vector.tensor_mul(sq[:], xc[:], xc[:])
            vs = work.tile([P, 1], F32, name="vs", tag="vs", bufs=2)
            nc.vector.tensor_reduce(vs[:], sq[:], axis=AX.X, op=OP.add)
            sd = work.tile([P, 1], F32, name="sd", tag="sd", bufs=2)
            nc.scalar.activation(sd[:], vs[:], AF.Sqrt, bias=1e-5, scale=1.0 / H)
            isd = work.tile([P, 1], F32, name=f"isd{c}", tag=f"isd{c}", bufs=1)
            nc.vector.reciprocal_approx_fast(isd[:], sd[:])
            xcs.append(xc)
            isds.append(isd)

        g1s = []
        for c in range(NCH):
            xn = work.tile([P, H], F32, name="xn", tag="xn", bufs=2)
            nc.vector.scalar_tensor_tensor(xn[:], in0=xcs[c][:], scalar=isds[c][:],
                                           in1=lngb[:], op0=OP.mult, op1=OP.mult)
            xg = work.tile([P, H], F32, name="xg", tag="xg", bufs=2)
            nc.vector.tensor_add(xg[:], xn[:], lnbb[:])
            g1 = work.tile([P, H], BF16, name=f"g1_{c}", tag=f"g1_{c}", bufs=1)
            nc.scalar.activation(g1[:], xg[:], AF.Gelu)
            g1s.append(g1)

        h2s = []
        for c in range(NCH):
            g1T = []
            for k in range(2):
                ptp = psA.tile([P, P], BF16, name="ptp", tag="tp16", bufs=2)
                nc.tensor.transpose(ptp[:], g1s[c][:, k * P:(k + 1) * P], ident16[:])
                t = work.tile([P, P], BF16, name=f"g1T{k}", tag=f"g1T{k}", bufs=2)
                nc.scalar.copy(t[:], ptp[:])
                g1T.append(t)
            ph2 = psA.tile([P, H // 2], F32, name="ph2", tag="h2", bufs=2)
            nc.tensor.matmul(ph2[:], g1T[0][:], w2s[0][:], start=True, stop=False)
            nc.tensor.matmul(ph2[:], g1T[1][:], w2s[1][:], start=False, stop=True)
            h2 = work.tile([P, H // 2], F32, name=f"h2_{c}", tag=f"h2_{c}", bufs=1)
            nc.vector.tensor_add(h2[:], ph2[:], b2b[:])
            h2s.append(h2)

        g2s = []
        for c in range(NCH):
            g2 = work.tile([P, H // 2], BF16, name=f"g2_{c}", tag=f"g2_{c}", bufs=1)
            nc.scalar.activation(g2[:], h2s[c][:], AF.Gelu)
            g2s.append(g2)

        for c in range(NCH):
            ptp = psA.tile([P, P], BF16, name="ptp3", tag="tp16", bufs=2)
            nc.tensor.transpose(ptp[:], g2s[c][:], ident16[:])
            g2T = work.tile([P, P], BF16, name="g2T", tag="g2T", bufs=2)
            nc.scalar.copy(g2T[:], ptp[:])
            nc.tensor.matmul(pe_[:, c:c + 1], g2T[:], w3s[:], start=True, stop=True)

        # errors: softplus (exp->ln) -> log1p -> robust norm
        ex3 = work.tile([P, NCH], F32, name="ex3", tag="ex3")
        nc.scalar.activation(ex3[:], pe_[:], AF.Exp, bias=b3c[:, 0:1])
        sp = work.tile([P, NCH], F32, name="sp", tag="sp")
        nc.scalar.activation(sp[:], ex3[:], AF.Ln, bias=1.0)
        el = work.tile([P, NCH], F32, name="el", tag="el")
        nc.scalar.activation(el[:], sp[:], AF.Ln, bias=1.0)

        mn_r = work.tile([P, 1], F32, name="mn_r", tag="mn_r")
        nc.vector.tensor_reduce(mn_r[:], el[:], axis=AX.X, op=OP.min)
        mn_neg = work.tile([P, 1], F32, name="mn_neg", tag="mn_neg")
        nc.vector.tensor_scalar_mul(mn_neg[:], mn_r[:], -1.0)
        mx_r = work.tile([P, 1], F32, name="mx_r", tag="mx_r")
        nc.vector.tensor_reduce(mx_r[:], el[:], axis=AX.X, op=OP.max)
        mn_negb = work.tile([P, 1], F32, name="mn_negb", tag="mn_negb")
        nc.gpsimd.partition_all_reduce(mn_negb[:], mn_neg[:], channels=P,
                                       reduce_op=bass_rust.ReduceOp.max)
        mx_b = work.tile([P, 1], F32, name="mx_b", tag="mx_b")
        nc.gpsimd.partition_all_reduce(mx_b[:], mx_r[:], channels=P,
                                       reduce_op=bass_rust.ReduceOp.max)
        rngc = work.tile([P, 1], F32, name="rngc", tag="rngc")
        nc.vector.tensor_add(rngc[:], mx_b[:], mn_negb[:])
        nc.vector.tensor_scalar_max(rngc[:], rngc[:], 1e-6)
        irng = work.tile([P, 1], F32, name="irng", tag="irng")
        nc.vector.reciprocal_approx_fast(irng[:], rngc[:])
        eln = work.tile([P, NCH], F32, name="eln", tag="eln")
        nc.vector.tensor_scalar(eln[:], in0=el[:], scalar1=mn_negb[:],
                                scalar2=irng[:], op0=OP.add, op1=OP.mult)
        s1 = work.tile([P, 1], F32, name="s1", tag="s1")
        nc.vector.tensor_reduce(s1[:], eln[:], axis=AX.X, op=OP.add)
        sall = work.tile([P, 1], F32, name="sall", tag="sall")
        nc.gpsimd.partition_all_reduce(sall[:], s1[:], channels=P,
                                       reduce_op=bass_rust.ReduceOp.add)
        meanb = work.tile([P, 1], F32, name="meanb", tag="meanb")
        nc.vector.tensor_scalar_mul(meanb[:], sall[:], 1.0 / L)
        # anomS = 8*(eln - mean);  strength = 1 - eln
        nc.vector.tensor_scalar(anomS[:], in0=eln[:], scalar1=meanb[:],
                                scalar2=8.0, op0=OP.subtract, op1=OP.mult)
        nc.vector.tensor_scalar(strength[:], in0=eln[:], scalar1=-1.0,
                                scalar2=1.0, op0=OP.mult, op1=OP.add)

    if STAGE < 2:
        nc.sync.dma_start(out=out_d.rearrange("(c p) t -> p c t", p=P),
                          in_=P_sb[:].rearrange("p (c t) -> p c t", t=2))
        nc.sync.dma_start(out=io["dbg"], in_=anomS[:])
        return

    # ---------------- stage B: pairwise phases ----------------
    Pv = P_sb[:].rearrange("p (c t) -> p c t", t=2)
    Pw16v = Pw16[:].rearrange("p (c t) -> p c t", t=3)

    with tc.tile_pool(name="psumB", bufs=1, space="PSUM") as psB:

        def build_XY():
            """Transpose current positions into broadcast rows xb/yb and
            refresh P_neg (square biases) + Pw16 (bf16 reduction weights)."""
            for half in range(2):
                ptx = psB.tile([2, 512], F32, name="ptx", tag="tpxy", bufs=2)
                for q in range(4):
                    c = half * 4 + q
                    nc.tensor.transpose(ptx[:, q * P:(q + 1) * P],
                                        P_sb[:, 2 * c:2 * c + 2], identity[:])
                nc.scalar.copy(XY[:, half * 512:(half + 1) * 512], ptx[:])
            nc.sync.dma_start(out=yrow[:], in_=XY[1:2, :])
            nc.gpsimd.partition_broadcast(xb[:], XY[0:1, :])
            nc.gpsimd.partition_broadcast(yb[:], yrow[:])
            nc.vector.tensor_scalar_mul(P_neg[:], P_sb[:], -1.0)
            nc.vector.tensor_copy(Pw16v[:, :, 0:2], Pv)

        def diag_zero(tile_ap):
            nc.gpsimd.affine_select(out=tile_ap, in_=tile_ap,
                                    compare_op=OP.not_equal, fill=0.0,
                                    base=0, pattern=[[-1, P]],
                                    channel_multiplier=1)

        def reductions(fields, acc, radius, los):
            # region-outer ordering: each PSUM region's accumulation group is
            # contiguous (matmul `start` clears has_written bank-wide)
            for ic in range(NCH):
                lo, hi = _win(ic, radius)
                for c in range(lo, hi):
                    off = (ic - los[c]) * P
                    nc.tensor.matmul(acc[:, 3 * ic:3 * ic + 3],
                                     fields[c][:, off:off + P],
                                     Pw16[:, 3 * c:3 * c + 3],
                                     start=(c == lo), stop=(c == hi - 1))

        # ======== phase 1: gravity + repulsion forces ========
        build_XY()
        # sweep 1 (sqrt table): d2 squares + dist; dist2_c tiles persist
        dist2s = []
        for c in range(NCH):
            sqx = work.tile([P, L], F32, name="sqx", tag="sqx", bufs=2)
            nc.scalar.activation(sqx[:], xb[:], AF.Square,
                                 bias=P_neg[:, 2 * c:2 * c + 1])
            sqy = work.tile([P, L], F32, name="sqy", tag="sqy", bufs=2)
            nc.scalar.activation(sqy[:], yb[:], AF.Square,
                                 bias=P_neg[:, 2 * c + 1:2 * c + 2])
            d2 = work.tile([P, L], F32, name="d2", tag="d2s", bufs=2)
            nc.vector.tensor_add(d2[:], sqx[:], sqy[:])
            # dist2 = 2*sqrt(d2+1e-12)
            dist2 = work.tile([P, L], F32, name=f"dist2_{c}", tag=f"dist2_{c}",
                              bufs=1)
            nc.scalar.activation(dist2[:], d2[:], AF.Sqrt, bias=4e-12, scale=4.0)
            dist2s.append(dist2)

        # sweep 2 (exp table + DVE field chain)
        fields = []
        for c in range(NCH):
            # exp(1 - d/DANGER), unclamped (relu folded into max below)
            e16 = work.tile([P, L], BF16, name="e16", tag="e16", bufs=2)
            nc.scalar.activation(e16[:], dist2s[c][:], AF.Exp,
                                 bias=1.0, scale=-0.5 / DANGER)
            iv5 = work.tile([P, L], F32, name="iv5", tag="iv5", bufs=2)
            nc.vector.reciprocal_approx_fast(iv5[:], dist2s[c][:])   # = 0.5/d
            inv2 = work.tile([P, L], BF16, name="inv2", tag="inv2", bufs=2)
            nc.vector.tensor_mul(inv2[:], iv5[:], iv5[:])        # 0.25/d^2
            inv3 = work.tile([P, L], BF16, name="inv3", tag="inv3", bufs=2)
            nc.vector.tensor_mul(inv3[:], inv2[:], iv5[:])       # 0.125/d^3
            # q = anom/d^3  (anomS = 8*anom; tiny for far pairs)
            q = work.tile([P, L], BF16, name="q", tag="q", bufs=2)
            nc.vector.tensor_scalar_mul(q[:], inv3[:], anomS[:, c:c + 1])
            # zz = max(e,1)-1  (exactly 0 beyond the danger radius)
            zz = work.tile([P, L], BF16, name="zz", tag="zz", bufs=2)
            nc.vector.tensor_scalar(zz[:], in0=e16[:], scalar1=1.0,
                                    scalar2=-1.0, op0=OP.max, op1=OP.add)
            # z = (max(e,1)-1) * 0.5/d  ( = REPULSION*(exp(relu(u))-1)/d )
            z = work.tile([P, L], BF16, name="z", tag="z", bufs=2)
            nc.vector.tensor_mul(z[:], zz[:], iv5[:])
            T16 = work.tile([P, L], BF16, name=f"T16_{c}", tag=f"T16_{c}", bufs=1)
            nc.vector.tensor_sub(T16[:], q[:], z[:])
            diag_zero(T16[:, c * P:(c + 1) * P])
            fields.append(T16)

        acc = psB.tile([P, 3 * NCH], F32, name="acc1", tag="acc")
        reductions(fields, acc, None, [0] * NCH)

        # ---- phase 1 epilogue: force -> displacement -> P_sb update
        accv = acc[:].rearrange("p (c t) -> p c t", t=3)
        t1 = work.tile([P, 2 * NCH], F32, name="t1", tag="ep16a")
        nc.vector.tensor_mul(
            t1[:].rearrange("p (c t) -> p c t", t=2), Pv,
            accv[:, :, 2:3].broadcast_to([P, NCH, 2]))
        F = work.tile([P, 2 * NCH], F32, name="F", tag="ep16b")
        nc.vector.tensor_sub(F[:].rearrange("p (c t) -> p c t", t=2),
                             accv[:, :, 0:2],
                             t1[:].rearrange("p (c t) -> p c t", t=2))
        sqF = work.tile([P, 2 * NCH], F32, name="sqF", tag="ep16a")
        nc.vector.tensor_mul(sqF[:], F[:], F[:])
        m2 = work.tile([P, NCH], F32, name="m2", tag="ep8a")
        nc.vector.tensor_reduce(m2[:], sqF[:].rearrange("p (c t) -> p c t", t=2),
                                axis=AX.X, op=OP.add)
        mag = work.tile([P, NCH], F32, name="mag", tag="ep8b")
        nc.scalar.activation(mag[:], m2[:], AF.Sqrt, bias=1e-16)
        msum = work.tile([P, 1], F32, name="msum", tag="msum")
        nc.vector.tensor_reduce(msum[:], mag[:], axis=AX.X, op=OP.add)
        msall = work.tile([P, 1], F32, name="msall", tag="msall")
        nc.gpsimd.partition_all_reduce(msall[:], msum[:], channels=P,
                                       reduce_op=bass_rust.ReduceOp.add)
        mmb = work.tile([P, 1], F32, name="mmb", tag="mmb")
        nc.vector.tensor_scalar(mmb[:], in0=msall[:], scalar1=1.0 / L,
                                scalar2=1e-8, op0=OP.mult, op1=OP.add)
        rmb = work.tile([P, 1], F32, name="rmb", tag="rmb")
        nc.vector.reciprocal_approx_fast(rmb[:], mmb[:])
        rel = work.tile([P, NCH], F32, name="rel", tag="ep8a")
        nc.vector.tensor_scalar_mul(rel[:], mag[:], rmb[:])
        dmp = work.tile([P, NCH], F32, name="dmp", tag="ep8c")
        nc.vector.tensor_scalar(dmp[:], in0=rel[:], scalar1=2.0,
                                scalar2=(MAX_DISP - MIN_DISP) / 2.0,
                                op0=OP.min, op1=OP.mult)
        den = work.tile([P, NCH], F32, name="den", tag="ep8a")
        nc.vector.tensor_scalar_add(den[:], mag[:], 1e-8)
        dn = work.tile([P, NCH], F32, name="dn", tag="ep8b")
        nc.vector.reciprocal_approx_fast(dn[:], den[:])
        uu = work.tile([P, NCH], F32, name="uu", tag="ep8a")
        nc.vector.scalar_tensor_tensor(uu[:], in0=dmp[:], scalar=MIN_DISP,
                                       in1=dn[:], op0=OP.add, op1=OP.mult)
        vv = work.tile([P, 2 * NCH], F32, name="vv", tag="ep16a")
        nc.vector.tensor_mul(vv[:].rearrange("p (c t) -> p c t", t=2),
                             F[:].rearrange("p (c t) -> p c t", t=2),
                             uu[:].unsqueeze(2).broadcast_to([P, NCH, 2]))
        pnew = work.tile([P, 2 * NCH], F32, name="pnew", tag="ep16b")
        nc.vector.tensor_add(pnew[:], P_sb[:], vv[:])
        nc.vector.tensor_scalar(P_sb[:], in0=pnew[:], scalar1=SMIN,
                                scalar2=SMAX, op0=OP.max, op1=OP.min)
        nc.vector.tensor_copy(P_start[:], P_sb[:])

        if STAGE < 3:
            nc.sync.dma_start(out=io["dbg"], in_=anomS[:])
            nc.sync.dma_start(out=out_d.rearrange("(c p) t -> p c t", p=P),
                              in_=P_sb[:].rearrange("p (c t) -> p c t", t=2))
            return

        # ======== phase 2: density spreading ========
        los = [_win(c, radius)[0] for c in range(NCH)]
        for it in range(DENSITY_ITERS):
            build_XY()
            d2bs = []
            for c in range(NCH):
                lo, hi = _win(c, radius)
                width = (hi - lo) * P
                sqx = work.tile([P, 5 * P], F32, name="sqxd", tag="sqx", bufs=2)
                nc.scalar.activation(sqx[:, 0:width], xb[:, lo * P:hi * P],
                                     AF.Square, bias=P_neg[:, 2 * c:2 * c + 1])
                sqy = work.tile([P, 5 * P], F32, name="sqyd", tag="sqy", bufs=2)
                nc.scalar.activation(sqy[:, 0:width], yb[:, lo * P:hi * P],
                                     AF.Square,
                                     bias=P_neg[:, 2 * c + 1:2 * c + 2])
                d2b = work.tile([P, 5 * P], BF16, name="d2b", tag=f"d2b_{c}",
                                bufs=1)
                nc.vector.tensor_add(d2b[:, 0:width], sqx[:, 0:width],
                                     sqy[:, 0:width])
                d2bs.append(d2b)
            fields = []
            for c in range(NCH):
                lo, hi = _win(c, radius)
                width = (hi - lo) * P
                wt = work.tile([P, 5 * P], BF16, name="wt", tag=f"wt_{c}", bufs=1)
                nc.scalar.activation(wt[:, 0:width], d2bs[c][:, 0:width],
                                     AF.Exp, scale=-S2)
                diag_zero(wt[:, (c - lo) * P:(c - lo + 1) * P])
                fields.append(wt)
            acc2 = psB.tile([P, 3 * NCH], F32, name=f"acc2_{it}", tag="acc")
            reductions(fields, acc2, radius, los)

            # epilogue: gradient -> clamped step -> clamped total -> clip
            accv = acc2[:].rearrange("p (c t) -> p c t", t=3)
            tg = work.tile([P, 2 * NCH], F32, name="tg", tag="ep16a")
            nc.vector.tensor_mul(tg[:].rearrange("p (c t) -> p c t", t=2), Pv,
                                 accv[:, :, 2:3].broadcast_to([P, NCH, 2]))
            ug = work.tile([P, 2 * NCH], F32, name="ug", tag="ep16b")
            nc.vector.tensor_sub(ug[:].rearrange("p (c t) -> p c t", t=2),
                                 tg[:].rearrange("p (c t) -> p c t", t=2),
                                 accv[:, :, 0:2])
            s_pre = work.tile([P, 2 * NCH], F32, name="s_pre", tag="ep16c")
            nc.vector.scalar_tensor_tensor(
                s_pre[:].rearrange("p (c t) -> p c t", t=2),
                in0=ug[:].rearrange("p (c t) -> p c t", t=2),
                scalar=STEP * 2.0 * S2,
                in1=strength[:].unsqueeze(2).broadcast_to([P, NCH, 2]),
                op0=OP.mult, op1=OP.mult)
            sqs = work.tile([P, 2 * NCH], F32, name="sqs", tag="ep16a")
            nc.vector.tensor_mul(sqs[:], s_pre[:], s_pre[:])
            sm2 = work.tile([P, NCH], F32, name="sm2", tag="ep8a")
            nc.vector.tensor_reduce(sm2[:],
                                    sqs[:].rearrange("p (c t) -> p c t", t=2),
                                    axis=AX.X, op=OP.add)
            smag = work.tile([P, NCH], F32, name="smag", tag="ep8b")
            nc.scalar.activation(smag[:], sm2[:], AF.Sqrt, bias=1e-16)
            sden = work.tile([P, NCH], F32, name="sden", tag="ep8a")
            nc.vector.tensor_scalar_add(sden[:], smag[:], 1e-8)
            sr = work.tile([P, NCH], F32, name="sr", tag="ep8b")
            nc.vector.reciprocal_approx_fast(sr[:], sden[:])
            sc = work.tile([P, NCH], F32, name="sc", tag="ep8a")
            nc.vector.tensor_scalar(sc[:], in0=sr[:], scalar1=MAX_STEP,
                                    scalar2=1.0, op0=OP.mult, op1=OP.min)
            sstep = work.tile([P, 2 * NCH], F32, name="sstep", tag="ep16a")
            nc.vector.tensor_mul(sstep[:].rearrange("p (c t) -> p c t", t=2),
                                 s_pre[:].rearrange("p (c t) -> p c t", t=2),
                                 sc[:].unsqueeze(2).broadcast_to([P, NCH, 2]))
            pn2 = work.tile([P, 2 * NCH], F32, name="pn2", tag="ep16b")
            nc.vector.tensor_add(pn2[:], P_sb[:], sstep[:])
            tot = work.tile([P, 2 * NCH], F32, name="tot", tag="ep16c")
            nc.vector.tensor_sub(tot[:], pn2[:], P_start[:])
            sqt = work.tile([P, 2 * NCH], F32, name="sqt", tag="ep16a")
            nc.vector.tensor_mul(sqt[:], tot[:], tot[:])
            tm2 = work.tile([P, NCH], F32, name="tm2", tag="ep8a")
            nc.vector.tensor_reduce(tm2[:],
                                    sqt[:].rearrange("p (c t) -> p c t", t=2),
                                    axis=AX.X, op=OP.add)
            tmag = work.tile([P, NCH], F32, name="tmag", tag="ep8b")
            nc.scalar.activation(tmag[:], tm2[:], AF.Sqrt, bias=1e-16)
            tden = work.tile([P, NCH], F32, name="tden", tag="ep8a")
            nc.vector.tensor_scalar_add(tden[:], tmag[:], 1e-8)
            tr = work.tile([P, NCH], F32, name="tr", tag="ep8b")
            nc.vector.reciprocal_approx_fast(tr[:], tden[:])
            tsc = work.tile([P, NCH], F32, name="tsc", tag="ep8a")
            nc.vector.tensor_scalar(tsc[:], in0=tr[:], scalar1=MAX_TOT,
                                    scalar2=1.0, op0=OP.mult, op1=OP.min)
            tot2 = work.tile([P, 2 * NCH], F32, name="tot2", tag="ep16a")
            nc.vector.tensor_mul(tot2[:].rearrange("p (c t) -> p c t", t=2),
                                 tot[:].rearrange("p (c t) -> p c t", t=2),
                                 tsc[:].unsqueeze(2).broadcast_to([P, NCH, 2]))
            pfin = work.tile([P, 2 * NCH], F32, name="pfin", tag="ep16b")
            nc.vector.tensor_add(pfin[:], P_start[:], tot2[:])
            nc.vector.tensor_scalar(P_sb[:], in0=pfin[:], scalar1=SMIN,
                                    scalar2=SMAX, op0=OP.max, op1=OP.min)

    # ---------------- output DMA ----------------
    nc.sync.dma_start(
        out=out_d.rearrange("(c p) t -> p c t", p=P),
        in_=P_sb[:].rearrange("p (c t) -> p c t", t=2),
    )


_PROGRAM_CACHE = {}


def _get_program(radius):
    key = ("nc", radius, STAGE)
    if key in _PROGRAM_CACHE:
        return _PROGRAM_CACHE[key]
    nc = bacc.Bacc("TRN2", target_bir_lowering=False, debug=False)
    # register the constant activation biases used below (only 0.0/1.0 ship)
    for v in (1e-5, 4e-12, 1e-16, 1e-8):
        t = nc.alloc_sbuf_tensor(f"const-f32-{v}", [128, 1], F32)
        nc.gpsimd.memset(t.ap(), v)
        nc.const_aps.aps[(F32, v)] = t.ap()
    nc.all_engine_barrier()
    io = {
        "latT": nc.dram_tensor("latT", [D, L], BF16, kind="ExternalInput").ap(),
        "positions": nc.dram_tensor("positions", [L, 2], F32, kind="ExternalInput").ap(),
        "w1": nc.dram_tensor("w1", [D, H], BF16, kind="ExternalInput").ap(),
        "b1": nc.dram_tensor("b1", [H], F32, kind="ExternalInput").ap(),
        "ln_g": nc.dram_tensor("ln_g", [H], F32, kind="ExternalInput").ap(),
        "ln_b": nc.dram_tensor("ln_b", [H], F32, kind="ExternalInput").ap(),
        "w2": nc.dram_tensor("w2", [H, H // 2], BF16, kind="ExternalInput").ap(),
        "b2": nc.dram_tensor("b2", [H // 2], F32, kind="ExternalInput").ap(),
        "w3": nc.dram_tensor("w3", [H // 2, 1], BF16, kind="ExternalInput").ap(),
        "b3": nc.dram_tensor("b3", [1], F32, kind="ExternalInput").ap(),
        "out": nc.dram_tensor("out", [L, 2], F32, kind="ExternalOutput").ap(),
        "dbg": nc.dram_tensor("dbg", [P, NCH], F32, kind="ExternalOutput").ap(),
    }
    with tile.TileContext(nc) as tc, ExitStack() as ctx:
        _build_kernel(ctx, tc, io, radius)
    nc.compile()
    _PROGRAM_CACHE[key] = nc
    return nc


def _grid_ok(positions):
    """Density chunk-skip is only valid for grid-like positions."""
    axis = np.linspace(SMIN, SMAX, N_ROW, dtype=np.float32)
    gy, gx = np.meshgrid(axis, axis, indexing="ij")
    grid = np.stack([gy.ravel(), gx.ravel()], axis=-1)  # [L, 2]
    jit = positions - grid[None, :, :]
    return float(np.abs(jit).max()) <= 2.0


def run(inputs, trace=False, **kwargs):
    bf16 = mybir.dt.np(BF16)
    radius = DENS_R if _grid_ok(np.asarray(inputs["positions"], np.float32)) else None
    nc = _get_program(radius)
    core_ids = list(range(B))
    shared = {k: np.ascontiguousarray(inputs[k], dtype=np.float32)
              for k in ("b1", "ln_g", "ln_b", "b2", "b3")}
    shared["w1"] = np.ascontiguousarray(inputs["w1"]).astype(bf16)
    shared["w2"] = np.ascontiguousarray(inputs["w2"]).astype(bf16)
    shared["w3"] = np.ascontiguousarray(inputs["w3"]).astype(bf16)
    in_maps = []
    for b in range(B):
        m = dict(shared)
        m["latT"] = np.ascontiguousarray(
            np.asarray(inputs["latents"][b], np.float32).T).astype(bf16)
        m["positions"] = np.ascontiguousarray(inputs["positions"][b],
                                              dtype=np.float32)
        in_maps.append(m)
    res = run_bass_kernel_spmd(nc, in_maps, core_ids, trace=trace, **kwargs)
    out = np.stack([res.results[b]["out"] for b in range(B)], axis=0)
    return out, res


def kernel(**inputs) -> np.ndarray:
    out, _ = run(inputs)
    return out
